# revision 15
# baseline (speedup 1.0000x reference)
"""Int8 quantize-dequantize Conv2d (3x3, pad 1) forward — Trainium2 Bass kernel.

Strategy: data-parallel over batch (4 images per core x 8 cores).
 - SBUF partitions = (image, in_channel): each partition holds one zero-padded
   (H+2)x(W+2) image plane.
 - Per-tensor symmetric int8 quantization: per-core abs-max reduced on DVE while
   DMA-in streams, GPSIMD partition all-reduce, then a 4-byte AllReduce(max)
   collective for the global x scale.  q = round(x/s) uses the +/-1.5*2^23
   magic-constant trick (exact round-half-to-even, matches jnp.round).
 - q values (|q| <= 127) are exact in bf16; fp32 PSUM accumulation of the
   3x3x32 contraction is exact integer arithmetic.
 - The 3x3 conv = 9 taps; each tap is a pure free-dim offset into the padded
   plane.  Each tap is a K=32(cin) x M=32(cout) matmul; tile_position packs
   16 concurrent 32x32 PE sub-arrays (4 images x 4 col-slots), 9 taps
   accumulate into each PSUM region.  N=486 (=3 padded rows) per matmul.
 - Drain: out = psum * (s_x*s_w) + bias on ACT/DVE, f32 staging tiles, then
   contiguous-burst DMA to NCHW HBM layout.
"""

import numpy as np

MAGIC = float(np.float32(1.5 * 2.0**23))  # 12582912.0
QMAX = 127.0

FULL_CFG = dict(NL=4, C=32, O=32, H=160, W=160, n_cores=8)

_CACHE = {}


def _build(cfg):
    import concourse.bacc as bacc
    import concourse.bass_isa as bass_isa
    import concourse.mybir as mybir
    import concourse.tile as tile

    NL, C, O, H, W = cfg["NL"], cfg["C"], cfg["O"], cfg["H"], cfg["W"]
    n_cores = cfg["n_cores"]
    HP, WP = H + 2, W + 2
    PLANE = HP * WP
    NPART = NL * C
    assert NPART <= 128 and C == 32 and O == 32

    NG = -(-H // 3)  # number of 3-row groups (last may be short)
    rows_last = H - 3 * (NG - 1)
    FULL_N = 3 * WP
    LAST_N = (rows_last - 1) * WP + W
    assert FULL_N <= 512
    NSLOT = 4
    n_phases = -(-NG // NSLOT)

    fp32 = mybir.dt.float32
    bf16 = mybir.dt.bfloat16
    AX = mybir.AxisListType.X
    AF = mybir.ActivationFunctionType
    ALU = mybir.AluOpType

    nc = bacc.Bacc("TRN2", target_bir_lowering=False, debug=False,
                   num_devices=n_cores)

    x_t = nc.dram_tensor("x", [NL, C, H, W], fp32, kind="ExternalInput")
    w_t = nc.dram_tensor("weight", [O, C, 3, 3], fp32, kind="ExternalInput")
    b_t = nc.dram_tensor("bias", [O], fp32, kind="ExternalInput")
    out_t = nc.dram_tensor("out", [NL, O, H, W], fp32, kind="ExternalOutput")

    with tile.TileContext(nc) as tc:
        with (
            tc.tile_pool(name="big", bufs=1) as big,
            tc.tile_pool(name="small", bufs=1) as small,
            tc.tile_pool(name="qtmp_pool", bufs=3) as qtmp_pool,
            tc.tile_pool(name="stage_pool", bufs=8) as stage_pool,
            tc.tile_pool(name="psum_pool", bufs=2, space="PSUM") as psum_pool,
            tc.tile_pool(name="dram_pool", bufs=1, space="DRAM") as dram_pool,
        ):
            # ---------------- resident tiles ----------------
            xf = big.tile([NPART, H * W], fp32)      # raw f32 planes
            xq = big.tile([NPART, PLANE], bf16)      # quantized padded planes
            wf = small.tile([NPART, 9 * O], fp32)    # weight, (c, o, tap), x4 replicas
            wq = small.tile([NPART, 9 * O], bf16)
            wtmp = small.tile([NPART, 9 * O], fp32)
            biasb = small.tile([128, 1], fp32)
            acc = small.tile([NPART, 8], fp32)
            xmax_col = small.tile([NPART, 1], fp32)
            xmax_all = small.tile([NPART, 1], fp32)
            wmax_col = small.tile([NPART, 1], fp32)
            wmax_all = small.tile([NPART, 1], fp32)
            wmaxb = small.tile([128, 1], fp32)
            gmax_p0 = small.tile([1, 1], fp32)
            gmaxb = small.tile([128, 1], fp32)
            s_x = small.tile([128, 1], fp32)
            r_x = small.tile([128, 1], fp32)
            s_w = small.tile([128, 1], fp32)
            r_w = small.tile([128, 1], fp32)
            s_out = small.tile([128, 1], fp32)
            one_sb = small.tile([128, 1], fp32)
            negmagic = small.tile([128, 1], fp32)

            nc.vector.memset(negmagic, -MAGIC)
            nc.vector.memset(one_sb, 1.0)

            x_hbm = x_t.ap().rearrange("n c h w -> (n c) (h w)")

            # ---------------- load x + local abs-max ----------------
            NCHUNK = 8
            assert (H * W) % NCHUNK == 0
            CH = H * W // NCHUNK
            for k in range(NCHUNK):
                sl = slice(k * CH, (k + 1) * CH)
                nc.sync.dma_start(xf[:, sl], x_hbm[:, sl])
                nc.vector.reduce_max(acc[:, k:k + 1], xf[:, sl], axis=AX,
                                     apply_absolute_value=True)
            nc.vector.reduce_max(xmax_col, acc, axis=AX)
            nc.gpsimd.partition_all_reduce(xmax_all, xmax_col, channels=NPART,
                                           reduce_op=bass_isa.ReduceOp.max)

            # ---------------- global max across cores ----------------
            if n_cores > 1:
                cc_in = dram_pool.tile([1, 1], fp32)
                cc_out = dram_pool.tile([1, 1], fp32)
                nc.sync.dma_start(cc_in[:, :], xmax_all[0:1, 0:1])
                nc.gpsimd.collective_compute(
                    "AllReduce", ALU.max,
                    replica_groups=[list(range(n_cores))],
                    ins=[cc_in.opt()], outs=[cc_out.opt()],
                )
                nc.sync.dma_start(gmax_p0[:, :], cc_out[:, :])
            else:
                nc.vector.tensor_copy(gmax_p0[:, :], xmax_all[0:1, 0:1])
            nc.gpsimd.partition_broadcast(gmaxb, gmax_p0[0:1, 0:1])

            # ---------------- scales ----------------
            # s = max(|v|)/127 ; clamp 1e-12 ; r = 1/s  (reference semantics)
            nc.vector.tensor_scalar(s_x, gmaxb, float(np.float32(1.0 / 127.0)),
                                    None, op0=ALU.mult)
            nc.vector.tensor_scalar_max(s_x, s_x, 1e-12)
            nc.vector.reciprocal(r_x, s_x)

            # ---------------- weight load / quant (local, pre-collective) ----
            # wf layout: partition c (x NL replicas), free = (o, tap)
            w_src = w_t.ap().rearrange("o c kh kw -> c o (kh kw)")
            wf_v = wf.rearrange("p (o t) -> p o t", t=9)
            for i in range(NL):
                nc.sync.dma_start(wf_v[32 * i:32 * i + 32, :, :], w_src)
            nc.vector.reduce_max(wmax_col, wf, axis=AX,
                                 apply_absolute_value=True)
            nc.gpsimd.partition_all_reduce(wmax_all, wmax_col, channels=NPART,
                                           reduce_op=bass_isa.ReduceOp.max)
            nc.gpsimd.partition_broadcast(wmaxb, wmax_all[0:1, 0:1])
            nc.vector.tensor_scalar(s_w, wmaxb, float(np.float32(1.0 / 127.0)),
                                    None, op0=ALU.mult)
            nc.vector.tensor_scalar_max(s_w, s_w, 1e-12)
            nc.vector.reciprocal(r_w, s_w)
            nc.vector.tensor_scalar(wtmp, wf, r_w[0:NPART, :], MAGIC,
                                    op0=ALU.mult, op1=ALU.add)
            nc.scalar.activation(wq, wtmp, AF.Identity,
                                 bias=negmagic[0:NPART, :], scale=1.0)
            wq_v = wq.rearrange("p (o t) -> p o t", t=9)

            # s_out = s_x * s_w (needs global s_x)
            nc.vector.tensor_scalar(s_out, s_x, s_w, None, op0=ALU.mult)

            # ---------------- bias ----------------
            b_src = b_t.ap().unsqueeze(-1)
            for j in range(4):
                nc.sync.dma_start(biasb[32 * j:32 * j + 32, :], b_src)

            # ---------------- xq borders ----------------
            xq_v = xq.rearrange("p (h w) -> p h w", w=WP)
            nc.vector.memset(xq_v[:, 0:1, :], 0.0)            # top row
            nc.vector.memset(xq_v[:, HP - 1:HP, :], 0.0)      # bottom row
            nc.vector.memset(xq_v[:, 1:HP - 1, 0:1], 0.0)     # left col
            nc.vector.memset(xq_v[:, 1:HP - 1, WP - 1:WP], 0.0)  # right col

            # ---------------- quantize x (chunked rows) ----------------
            CHR = 16
            xf_v = xf.rearrange("p (h w) -> p h w", w=W)
            h0q = 0
            while h0q < H:
                nr = min(CHR, H - h0q)
                qtmp = qtmp_pool.tile([NPART, CHR * W], fp32, tag="qtmp",
                                      name=f"qtmp_{h0q}")
                qt = qtmp[:, 0:nr * W]
                nc.vector.tensor_scalar(qt, xf[:, h0q * W:(h0q + nr) * W],
                                        r_x[0:NPART, :], MAGIC,
                                        op0=ALU.mult, op1=ALU.add)
                qt_v = qtmp.rearrange("p (h w) -> p h w", w=W)[:, 0:nr, :]
                nc.scalar.activation(
                    xq_v[:, 1 + h0q:1 + h0q + nr, 1:1 + W], qt_v,
                    AF.Identity, bias=negmagic[0:NPART, :], scale=1.0)
                h0q += nr

            # ---------------- conv: 16-way tile_position matmuls ----------
            for ph in range(n_phases):
                slots = [(j, ph * NSLOT + j) for j in range(NSLOT)
                         if ph * NSLOT + j < NG]
                ps = {}
                for i in range(NL):
                    ps[i] = psum_pool.tile([128, FULL_N], fp32, tag=f"ps{i}",
                                           name=f"ps{i}_{ph}")
                # tap-major issue order: all 16 sub-array positions stream
                # concurrently; taps accumulate per PSUM region.  (HW probe
                # confirmed start=True clears has_written only for the
                # written partition slice, so interleaved col-tile groups in
                # one bank are safe.  CoreSim's pending-zero bookkeeping
                # can't model that — sim_safe serializes groups instead.)
                def emit_mm(i, j, g, tap):
                    kh, kw = divmod(tap, 3)
                    h0 = 3 * g
                    nf = FULL_N if g < NG - 1 else LAST_N
                    off = (h0 + kh) * WP + kw
                    nc.tensor.matmul(
                        ps[i][32 * j:32 * j + 32, 0:nf],
                        wq_v[32 * i:32 * i + 32, :, tap],
                        xq[32 * i:32 * i + 32, off:off + nf],
                        start=(tap == 0), stop=(tap == 8),
                        skip_group_check=True,
                        tile_position=(32 * i, 32 * j),
                    )

                if cfg.get("sim_safe"):
                    for i in range(NL):
                        for (j, g) in slots:
                            for tap in range(9):
                                emit_mm(i, j, g, tap)
                else:
                    for tap in range(9):
                        for i in range(NL):
                            for (j, g) in slots:
                                emit_mm(i, j, g, tap)
                # drain: out = psum * s_out + bias, interior columns only
                full_slots = [(j, g) for (j, g) in slots if g < NG - 1]
                nfull = len(full_slots)
                for i in range(NL):
                    if nfull:
                        p_hi = 32 * nfull
                        src = ps[i][0:p_hi, :].rearrange(
                            "p (r w) -> p r w", w=WP)[:, 0:3, 0:W]
                        stage = stage_pool.tile([128, 3 * W], fp32,
                                                tag="stage",
                                                name=f"stage_{ph}_{i}")
                        dst = stage[0:p_hi, :].rearrange(
                            "p (r w) -> p r w", w=W)
                        if (ph + i) % 2 == 0:
                            nc.scalar.activation(dst, src, AF.Identity,
                                                 bias=biasb[0:p_hi, :],
                                                 scale=s_out[0:p_hi, :])
                        else:
                            nc.vector.tensor_scalar(dst, src,
                                                    s_out[0:p_hi, :],
                                                    biasb[0:p_hi, :],
                                                    op0=ALU.mult, op1=ALU.add)
                        h_lo = 3 * (ph * NSLOT)
                        out_dst = out_t.ap()[i, :, h_lo:h_lo + 3 * nfull, :] \
                            .rearrange("o (j r) w -> j o r w", j=nfull)
                        nc.sync.dma_start(out_dst, stage[0:p_hi, :])
                    # short last group (if it lands in this phase)
                    for (j, g) in slots:
                        if g != NG - 1:
                            continue
                        p_lo, p_hi = 32 * j, 32 * j + 32
                        src = ps[i][p_lo:p_hi, :].rearrange(
                            "p (r w) -> p r w", w=WP)[:, 0:rows_last, 0:W]
                        stage2 = stage_pool.tile([128, 3 * W], fp32,
                                                 tag="stage",
                                                 name=f"stageL_{ph}_{i}")
                        dst = stage2[p_lo:p_hi, 0:rows_last * W].rearrange(
                            "p (r w) -> p r w", w=W)
                        if (ph + i) % 2 == 0:
                            nc.scalar.activation(dst, src, AF.Identity,
                                                 bias=biasb[p_lo:p_hi, :],
                                                 scale=s_out[p_lo:p_hi, :])
                        else:
                            nc.vector.tensor_scalar(dst, src,
                                                    s_out[p_lo:p_hi, :],
                                                    biasb[p_lo:p_hi, :],
                                                    op0=ALU.mult, op1=ALU.add)
                        h_lo = 3 * g
                        out_dst = out_t.ap()[i, :, h_lo:H, :].rearrange(
                            "o r w -> o (r w)")
                        nc.sync.dma_start(out_dst,
                                          stage2[p_lo:p_hi, 0:rows_last * W])

    nc.compile()
    return nc


def _get_nc(cfg_key=None):
    key = "full"
    if key not in _CACHE:
        _CACHE[key] = _build(FULL_CFG)
    return _CACHE[key]


def kernel(x, weight, bias):
    from concourse import bass_utils

    cfg = FULL_CFG
    NL, n_cores = cfg["NL"], cfg["n_cores"]
    nc = _get_nc()

    in_maps = []
    for c in range(n_cores):
        in_maps.append({
            "x": np.ascontiguousarray(x[c * NL:(c + 1) * NL]),
            "weight": np.ascontiguousarray(weight),
            "bias": np.ascontiguousarray(bias),
        })
    res = bass_utils.run_bass_kernel_spmd(nc, in_maps,
                                          core_ids=list(range(n_cores)))
    out = np.concatenate([r["out"] for r in res.results], axis=0)
    return out.astype(np.float32, copy=False)
